# revision 1
# baseline (speedup 1.0000x reference)
"""Causal self-attention with interleaved RoPE, tensor-parallel over heads on 8 NeuronCores.

Strategy (per core c, heads hA=2c, hB=2c+1):
  - All on-chip tensors live "transposed": feature dim on partitions, tokens on free dim.
  - QKV projection: psum[dcol, tok] = qkv_wT_tile.T @ xT_tile  (contraction over C in 8 tiles).
  - RoPE applied in-transposed layout: q_rot = q*cosT + swap(q*sinTt) where swap
    (adjacent-partition exchange) is a DVE stream_shuffle and the sin table is
    sign-folded + pair-reindexed on the host so the swap commutes.
  - Scores computed transposed: S^T[tk, tq] = K^T.T @ Q^T per head, two heads packed in
    the PE array via row tiling (contraction = Dh = 64 each).
  - softmax: exp on ACT (scale=1/8 folded in; key-mask bias per partition folded in);
    causal masking of diagonal blocks via host-precomputed mask multiply; row sums come
    free from an all-ones block col-packed into the AV matmul; normalization =
    reciprocal + multiply on DVE.
  - AV: y'^T[{d|r}, tq] += [V_h | ones].T @ E^T per tk tile (V transposed on PE;
    the ones half-block makes rows 64:128 the softmax denominator, pre-broadcast).
  - Out-projection: partial out^T[c_out, tq] = owT.T @ y_norm^T, DMA'd psum->HBM.
  - Host: gathers 8 partial outputs, sums, applies query mask and out bias.
Matmuls use float32r (1 cycle/row at N>=512): tensors feeding matmuls are declared
float32r end-to-end; host pre-rounds DMA'd data to the 11-mantissa-bit format.
"""

import numpy as np

B, T, C = 2, 2048, 1024
H, DH = 16, 64
NCORES = 8
CT = C // 128  # 8 contraction tiles
NEG = -1e30

_PROGRAM_CACHE = {}
LAST_RESULTS = None


def _build_program(has_qkv_bias=False):
    import concourse.mybir as mybir
    import concourse.tile as tile
    from concourse import bacc
    from contextlib import ExitStack

    F32 = mybir.dt.float32
    F32R = mybir.dt.float32r
    EXP = mybir.ActivationFunctionType.Exp

    SWAP_MASK = [i ^ 1 for i in range(32)]
    nc = bacc.Bacc("TRN2", target_bir_lowering=False, debug=False)

    # ---- DRAM I/O ----
    xT_d = nc.dram_tensor("xT", (CT, 128, B, T), F32R, kind="ExternalInput")
    qkvwT_d = nc.dram_tensor("qkvwT", (3, CT, 128, 128), F32R, kind="ExternalInput")
    bqkv_d = nc.dram_tensor("bqkv", (128, 3), F32, kind="ExternalInput")
    owT_d = nc.dram_tensor("owT", (8, 128, 128), F32R, kind="ExternalInput")
    cosT_d = nc.dram_tensor("cosT", (128, T), F32, kind="ExternalInput")
    sinTt_d = nc.dram_tensor("sinTt", (128, T), F32, kind="ExternalInput")
    triC_d = nc.dram_tensor("triC", (128, 2048), F32, kind="ExternalInput")
    vones_d = nc.dram_tensor("vones", (128, 256), F32R, kind="ExternalInput")
    ident_d = nc.dram_tensor("ident", (128, 128), F32, kind="ExternalInput")
    expb_d = nc.dram_tensor("expb", (128, 2 * (T // 128)), F32, kind="ExternalInput")
    outp_d = nc.dram_tensor("outp", (8, 128, B, T), F32, kind="ExternalOutput")

    NTK = T // 128  # 16 key tiles
    NJ = T // 512  # 4 query tiles

    with tile.TileContext(nc) as tc, ExitStack() as ctx:
        cpool = ctx.enter_context(tc.tile_pool(name="consts", bufs=1))
        xpool = ctx.enter_context(tc.tile_pool(name="xt", bufs=CT))
        spool = ctx.enter_context(tc.tile_pool(name="seq", bufs=2))
        vpool = ctx.enter_context(tc.tile_pool(name="vsb", bufs=20))
        epool = ctx.enter_context(tc.tile_pool(name="eexp", bufs=6))
        tpool = ctx.enter_context(tc.tile_pool(name="tmp", bufs=2))
        rpool = ctx.enter_context(tc.tile_pool(name="rr", bufs=2))
        spsum = ctx.enter_context(tc.tile_pool(name="S", bufs=2, space="PSUM"))
        qpool = ctx.enter_context(tc.tile_pool(name="qp", bufs=2, space="PSUM"))
        ypool = ctx.enter_context(tc.tile_pool(name="yp", bufs=2, space="PSUM"))

        def load_const(nm, dram_ap, shape, dt=F32):
            t = cpool.tile(shape, dt, name=nm, tag=nm)
            nc.sync.dma_start(t[:], dram_ap)
            return t

        qkvw = [
            [
                load_const(f"c_w{s}_{k}", qkvwT_d[s, k, :, :], [128, 128], F32R)
                for k in range(CT)
            ]
            for s in range(3)
        ]

        # dummy exp so the ACT table set loads during the initial DMA fill
        # instead of on the first real softmax tile
        warm = cpool.tile([128, 1], F32, name="warm", tag="warm")
        nc.vector.memset(warm[:], 0.0)
        nc.scalar.activation(warm[:], warm[:], EXP)

        def load_xt_half(b, half):
            xt = []
            for k in range(CT):
                t = xpool.tile([128, T // 2], F32R, tag="xt", name=f"xt{b}_{half}_{k}")
                nc.sync.dma_start(
                    t[:], xT_d[k, :, b, 1024 * half : 1024 * (half + 1)]
                )
                xt.append(t)
            return xt

        # DMA issue order = need order: first strip, rope tables, second strip,
        # transpose/attention consts, out weights, batch-1 strips.
        xts = {}
        xts[(0, 0)] = load_xt_half(0, 0)
        cosT = load_const("c_cos", cosT_d[:, :], [128, T])
        sinTt = load_const("c_sin", sinTt_d[:, :], [128, T])
        bqkv = load_const("c_bq", bqkv_d[:, :], [128, 3])
        ident = load_const("c_id", ident_d[:, :], [128, 128])
        vones = load_const("c_ones", vones_d[:, :], [128, 256], F32R)
        triC = load_const("c_tri", triC_d[:, :], [128, 2048])
        expb = load_const("c_eb", expb_d[:, :], [128, 2 * NTK])
        xts[(0, 1)] = load_xt_half(0, 1)
        owT = [
            load_const(f"c_ow{m}", owT_d[m, :, :], [128, 128], F32R) for m in range(8)
        ]
        xts[(1, 0)] = load_xt_half(1, 0)
        xts[(1, 1)] = load_xt_half(1, 1)

        for b in range(B):
            q2T = spool.tile([128, T], F32R, tag="q2T")
            k2T = spool.tile([128, T], F32R, tag="k2T")
            v2T = spool.tile([128, T], F32, tag="v2T")
            dsts = [q2T, k2T, v2T]

            # ---- QKV projection + RoPE, query-chunk-major so attention can
            # start as soon as the first (q,k,v) triple lands ----
            vsb = []
            for half in range(2):
                xt = xts[(b, half)]
                for jh in range(2):
                    jc = 2 * half + jh
                    sl = slice(512 * jc, 512 * (jc + 1))
                    xsl = slice(512 * jh, 512 * (jh + 1))
                    for s in range(3):
                        ps = qpool.tile([128, 512], F32, tag="qp")
                        for k in range(CT):
                            nc.tensor.matmul(
                                ps[:],
                                qkvw[s][k][:],
                                xt[k][:, xsl],
                                start=(k == 0),
                                stop=(k == CT - 1),
                            )
                        if has_qkv_bias:
                            nc.vector.tensor_scalar_add(
                                ps[:], ps[:], bqkv[:, s : s + 1]
                            )
                        if s == 2:
                            nc.vector.tensor_copy(v2T[:, sl], ps[:])
                        else:
                            t1 = tpool.tile([128, 512], F32, tag="t1")
                            t2 = tpool.tile([128, 512], F32, tag="t2")
                            t2s = tpool.tile([128, 512], F32, tag="t2s")
                            nc.vector.tensor_mul(t1[:], ps[:], cosT[:, sl])
                            nc.vector.tensor_mul(t2[:], ps[:], sinTt[:, sl])
                            nc.vector.stream_shuffle(t2s[:], t2[:], SWAP_MASK)
                            nc.gpsimd.tensor_add(dsts[s][:, sl], t1[:], t2s[:])
                    # transpose this chunk's V tiles: 4 PE transposes into one
                    # psum slot, then unpack into [V_A | 1s | V_B | 1s] tiles
                    vtg = qpool.tile([128, 512], F32, tag="qp", name=f"vtg{b}_{jc}")
                    for u in range(4):
                        t = 4 * jc + u
                        nc.tensor.transpose(
                            vtg[:, 128 * u : 128 * (u + 1)],
                            v2T[:, 128 * t : 128 * (t + 1)],
                            ident[:],
                        )
                    for u in range(4):
                        t = 4 * jc + u
                        vs = vpool.tile(
                            [128, 256], F32R, tag="vsb", name=f"vs{b}_{t}"
                        )
                        nc.gpsimd.tensor_copy(vs[:, 64:128], vones[:, 64:128])
                        nc.gpsimd.tensor_copy(vs[:, 192:256], vones[:, 192:256])
                        nc.vector.tensor_copy(
                            vs[:, 0:64], vtg[:, 128 * u : 128 * u + 64]
                        )
                        nc.vector.tensor_copy(
                            vs[:, 128:192], vtg[:, 128 * u + 64 : 128 * u + 128]
                        )
                        vsb.append(vs)

            # ---- attention (2 heads packed) ----
            y2T = spool.tile([128, T], F32R, tag="y2T")
            for j in range(NJ):
                jsl = slice(512 * j, 512 * (j + 1))
                yp = [
                    ypool.tile([128, 512], F32, tag="yp", name=f"yp{b}_{j}_{h}")
                    for h in range(2)
                ]
                ntk_j = 4 * (j + 1)
                for t in range(ntk_j):
                    tsl = slice(128 * t, 128 * (t + 1))
                    ecol = b * NTK + t
                    S = spsum.tile([128, 1024], F32, tag="S")
                    for h in range(2):
                        hsl = slice(64 * h, 64 * (h + 1))
                        nc.tensor.matmul(
                            S[:, 512 * h : 512 * (h + 1)],
                            k2T[hsl, tsl],
                            q2T[hsl, jsl],
                            start=True,
                            stop=True,
                            tile_position=(64 * h, 0),
                        )
                    E = epool.tile([128, 1024], F32R, tag="E")
                    m = t - 4 * j if t >= 4 * j else -1
                    if m >= 1:
                        # diagonal tile: skip exp over the fully-masked leading
                        # cols (zeroed explicitly), two-segment AP over both heads
                        for h in range(2):
                            nc.gpsimd.tensor_scalar_mul(
                                E[:, 512 * h : 512 * h + 128 * m],
                                triC[:, 0 : 128 * m],
                                0.0,
                            )
                        seg = E[:, 0:1024].rearrange(
                            "p (h c) -> p h c", h=2
                        )[:, :, 128 * m : 512]
                        sseg = S[:, 0:1024].rearrange(
                            "p (h c) -> p h c", h=2
                        )[:, :, 128 * m : 512]
                        nc.scalar.activation(
                            seg,
                            sseg,
                            EXP,
                            bias=expb[:, ecol : ecol + 1],
                            scale=0.125,
                        )
                    else:
                        nc.scalar.activation(
                            E[:],
                            S[:],
                            EXP,
                            bias=expb[:, ecol : ecol + 1],
                            scale=0.125,
                        )
                    if m >= 0:
                        for h in range(2):
                            nc.gpsimd.tensor_mul(
                                E[:, 512 * h + 128 * m : 512 * h + 128 * (m + 1)],
                                E[:, 512 * h + 128 * m : 512 * h + 128 * (m + 1)],
                                triC[:, 512 * m + 128 * m : 512 * m + 128 * (m + 1)],
                            )
                    last = t == ntk_j - 1
                    for h in range(2):
                        nc.tensor.matmul(
                            yp[h][:],
                            vsb[t][:, 128 * h : 128 * h + 128],
                            E[:, 512 * h : 512 * (h + 1)],
                            start=(t == 0),
                            stop=last,
                        )
                # normalize: y / rowsum, write into stacked y2T
                for h in range(2):
                    hsl = slice(64 * h, 64 * (h + 1))
                    rr = rpool.tile([64, 512], F32, tag="rr")
                    nc.vector.reciprocal(rr[:], yp[h][64:128, :])
                    nc.vector.tensor_mul(y2T[hsl, jsl], yp[h][0:64, :], rr[:])

                # ---- output projection for this query tile (deprioritized
                # so it fills gaps instead of blocking the next j's softmax) ----
                for mt in range(8):
                    op = qpool.tile(
                        [128, 512], F32, tag="qp", name=f"op{b}_{j}_{mt}"
                    )
                    nc.tensor.matmul(
                        op[:],
                        owT[mt][:],
                        y2T[:, jsl],
                        start=True,
                        stop=True,
                    )
                    ot = epool.tile(
                        [128, 512], F32, tag="ot", name=f"ot{b}_{j}_{mt}", bufs=3
                    )
                    if mt % 2 == 0:
                        nc.vector.tensor_copy(ot[:], op[:])
                    else:
                        nc.scalar.copy(ot[:], op[:])
                    nc.scalar.dma_start(outp_d[mt, :, b, jsl], ot[:])


    nc.compile()
    return nc


def _round_fp32r(a):
    """Round-to-nearest-even to fp32r (1s+8e+11m, value kept in the fp32 high bits)."""
    u = np.ascontiguousarray(a, np.float32).view(np.uint32)
    keep = u & np.uint32(0xFFFFF000)
    rem = u & np.uint32(0x00000FFF)
    lsb = (u >> np.uint32(12)) & np.uint32(1)
    up = (rem > 0x800) | ((rem == 0x800) & (lsb == 1))
    return (keep + (up.astype(np.uint32) << np.uint32(12))).view(np.float32)


def _host_inputs(x, attention_mask, qkv_w, qkv_b, out_w):
    """Build the device input tensors. Returns (shared dict, per-core list of dicts)."""
    x = np.ascontiguousarray(np.asarray(x, np.float32))
    qkv_w = np.asarray(qkv_w, np.float32)
    qkv_b = np.asarray(qkv_b, np.float32)
    out_w = np.asarray(out_w, np.float32)
    am = np.asarray(attention_mask)

    xT = _round_fp32r(x.transpose(2, 0, 1).reshape(CT, 128, B, T))

    # RoPE tables (match reference: interleaved rotate, concatenated freq table)
    inv_freq = 1.0 / (10000.0 ** (np.arange(0, DH, 2, dtype=np.float64) / DH))
    tt = np.arange(T, dtype=np.float64)
    freqs = np.outer(tt, inv_freq)  # [T, 32]
    emb = np.concatenate([freqs, freqs], axis=-1)  # [T, 64]
    cos = np.cos(emb).astype(np.float32).T  # [64, T]
    sin = np.sin(emb).astype(np.float32).T  # [64, T]
    sinTt64 = np.empty((DH, T), np.float32)
    sinTt64[0::2] = sin[1::2]  # sinTt[2i]   = +sin[2i+1]
    sinTt64[1::2] = -sin[0::2]  # sinTt[2i+1] = -sin[2i]
    cosT = np.ascontiguousarray(np.tile(cos, (2, 1)))  # [128, T]
    sinTt = np.ascontiguousarray(np.tile(sinTt64, (2, 1)))

    triC = np.zeros((128, 2048), np.float32)
    cc = np.arange(512)[None, :]
    pp = np.arange(128)[:, None]
    for m in range(4):
        triC[:, 512 * m : 512 * (m + 1)] = (cc >= 128 * m + pp).astype(np.float32)

    vones = np.ones((128, 256), np.float32)
    ident = np.eye(128, dtype=np.float32)

    ntk = T // 128
    key_ok = am.astype(bool).reshape(B, ntk, 128)  # [b, t, p]
    expb = np.where(key_ok, 0.0, NEG).astype(np.float32)
    expb = np.ascontiguousarray(expb.transpose(2, 0, 1).reshape(128, B * ntk))

    shared = dict(
        xT=xT, cosT=cosT, sinTt=sinTt, triC=triC,
        vones=vones, ident=ident, expb=expb,
    )

    per_core = []
    for c in range(NCORES):
        r0 = 128 * c
        qkvwT = _round_fp32r(
            np.stack(
                [
                    np.ascontiguousarray(
                        qkv_w[s * C + r0 : s * C + r0 + 128, :].T
                    ).reshape(CT, 128, 128)
                    for s in range(3)
                ]
            )
        )
        bqkv = np.stack(
            [qkv_b[s * C + r0 : s * C + r0 + 128] for s in range(3)], axis=1
        )  # [128, 3]
        ow_slice = out_w[:, r0 : r0 + 128]  # [1024, 128]
        owT = _round_fp32r(ow_slice.reshape(8, 128, 128).transpose(0, 2, 1))
        per_core.append(
            dict(
                qkvwT=qkvwT,
                bqkv=np.ascontiguousarray(bqkv),
                owT=owT,
            )
        )
    return shared, per_core


def kernel(x, attention_mask, qkv_w, qkv_b, out_w, out_b, _trace=False):
    global LAST_RESULTS
    from concourse.bass_utils import run_bass_kernel_spmd

    key = ("nc", bool(np.any(np.asarray(qkv_b))))
    if key not in _PROGRAM_CACHE:
        _PROGRAM_CACHE[key] = _build_program(has_qkv_bias=key[1])
    nc = _PROGRAM_CACHE[key]

    shared, per_core = _host_inputs(x, attention_mask, qkv_w, qkv_b, out_w)
    in_maps = [{**shared, **pc} for pc in per_core]

    res = run_bass_kernel_spmd(
        nc,
        in_maps,
        core_ids=list(range(NCORES)),
        trace=_trace,
        trace_cores=list(range(NCORES)) if _trace else None,
        stitch_traces=bool(_trace),
    )
    LAST_RESULTS = res

    acc = np.zeros((B, T, C), np.float64)
    for c in range(NCORES):
        part = res.results[c]["outp"]  # [8, 128, B, T]
        acc += part.transpose(2, 3, 0, 1).reshape(B, T, C)

    qm = np.asarray(attention_mask).astype(bool)
    out = np.where(qm[..., None], acc, 0.0) + np.asarray(out_b, np.float64)[None, None]
    return out.astype(np.float32)

